# revision 5
# baseline (speedup 1.0000x reference)
"""Trainium2 Bass kernel for nn_CorePartLayer.

Computes: proj = (L * z) @ U + mu  -> (B, DIM); reshaped to (B, C, 32, 32, 32)
and placed at offset 16 on each spatial axis inside a zero (B, C, 64, 64, 64)
output.

Sharding: one channel per NeuronCore (DIM = C * 32^3 and C == n_cores == 8).
Core c gets U[:, c*32768:(c+1)*32768] and mu[c*32768:(c+1)*32768], computes the
full-batch projection for its channel, and writes the padded (B, 64, 64, 64)
channel volume. Host stacks the 8 channel volumes into the final output.

Per-core dataflow:
  - z (32,64) DMA'd in, PE-transposed via identity matmul, scaled by L with a
    per-partition tensor_scalar, then augmented with a ones row so mu rides the
    matmul as contraction row 64 (K=65).
  - U streamed in 8 chunks of (65, 4096) — 4096 columns = 4 d-planes.
  - Per chunk: 8 fp32 matmuls (M=32, N=512) write a (128,512) PSUM tile at
    partition offsets 32j (PE array column tiling), so PSUM partition 32j+b
    holds plane j of batch b. Two DVE copies scatter the 32x32 interior rows
    into a pre-zeroed (128, 4096) padded-plane tile; one 2MB DMA stores it.
  - The 32 all-zero d-planes are stored from a persistent zero tile.
"""

from contextlib import ExitStack

import numpy as np

import concourse.bass as bass
import concourse.tile as tile
from concourse import bacc, mybir
from concourse.bass_utils import run_bass_kernel_spmd

B = 32          # batch
NB = 64         # n_basis (contraction)
C = 8           # channels == n_cores
CORE = 32       # core cube edge
RES = 64        # output cube edge
POS = 16        # placement offset
CPD = CORE * CORE * CORE  # columns per channel = 32768
PLANE = RES * RES         # 4096 floats per padded d-plane
GROUP = 4                 # d-planes per store group
NGROUPS = CORE // GROUP   # 8 interior groups
F32 = mybir.dt.float32

# Write the 32 all-zero d-planes explicitly. (The PJRT path donates
# pre-zeroed output buffers, but we do not rely on that.)
WRITE_ZERO_PLANES = True

_NC_CACHE = {}


def _emit(ctx, tc):
    nc = tc.nc
    z = nc.dram_tensor("z", [B, NB], F32, kind="ExternalInput").ap()
    Ld = nc.dram_tensor("L", [NB, 1], F32, kind="ExternalInput").ap()
    U = nc.dram_tensor("U", [NB, CPD], F32, kind="ExternalInput").ap()
    mu = nc.dram_tensor("mu", [CPD], F32, kind="ExternalInput").ap()
    out = nc.dram_tensor("out", [B, RES, PLANE], F32, kind="ExternalOutput").ap()

    const = ctx.enter_context(tc.tile_pool(name="const", bufs=1))
    upool = ctx.enter_context(tc.tile_pool(name="u", bufs=3))
    pads = ctx.enter_context(tc.tile_pool(name="pads", bufs=1))
    pzt = ctx.enter_context(tc.tile_pool(name="pzt", bufs=1, space="PSUM"))
    pmm = ctx.enter_context(tc.tile_pool(name="pmm", bufs=6, space="PSUM"))

    # --- lhsT prep: lhsT[k, b] = L[k] * z[b, k]; row NB is ones (mu row) ---
    z_t = const.tile([B, NB], F32, tag="z")
    L_t = const.tile([NB, 1], F32, tag="L")
    ones_t = const.tile([B, B], F32, tag="ones")
    id_t = const.tile([B, B], F32, tag="ident")
    lhsT = const.tile([NB + 1, B], F32, tag="lhsT")

    nc.sync.dma_start(z_t[:, :], z)
    nc.sync.dma_start(L_t[:, :], Ld)
    nc.vector.memset(ones_t[:, :], 1.0)
    # identity: iota(p - f) == 0 on the diagonal
    nc.gpsimd.affine_select(
        id_t[:, :],
        ones_t[:, :],
        pattern=[[-1, B]],
        compare_op=mybir.AluOpType.is_equal,
        fill=0.0,
        base=0,
        channel_multiplier=1,
    )
    zTp = pzt.tile([NB, B], F32, tag="zT")
    nc.tensor.transpose(zTp[:, :], z_t[:, :], id_t[:, :])
    nc.vector.tensor_scalar(
        lhsT[0:NB, :], zTp[:, :], L_t[0:NB, :], None, mybir.AluOpType.mult
    )
    nc.vector.memset(lhsT[NB : NB + 1, :], 1.0)

    # --- padded-plane buffers (zeros outside the 32x32 interior persist) ---
    zero_t = pads.tile([128, PLANE], F32, tag="zt")
    nc.vector.memset(zero_t[:, :], 0.0)
    NPAD = 3
    pad_ts = []
    for i in range(NPAD):
        t = pads.tile([128, PLANE], F32, tag=f"pad{i}")
        nc.vector.memset(t[:, :], 0.0)
        pad_ts.append(t)

    zero_d0 = [0, 4, 8, 12, 48, 52, 56, 60]

    for g in range(NGROUPS):
        # U chunk: 4096 columns = planes [4g, 4g+4) of the 32^3 block
        u_t = upool.tile([NB + 1, GROUP * 1024], F32, tag="u")
        c0 = g * GROUP * 1024
        nc.scalar.dma_start(u_t[0:NB, :], U[:, c0 : c0 + GROUP * 1024])
        nc.scalar.dma_start(u_t[NB : NB + 1, :], mu[c0 : c0 + GROUP * 1024])

        pA = pmm.tile([128, 512], F32, tag="mm")
        pB = pmm.tile([128, 512], F32, tag="mm")
        for j in range(GROUP):
            # PSUM partition 32j+b <- proj[b, plane 4g+j], halves of 1024 cols
            nc.tensor.matmul(
                pA[32 * j : 32 * j + 32, :],
                lhsT[:, :],
                u_t[:, j * 1024 : j * 1024 + 512],
                start=True,
                stop=True,
                tile_position=(0, 32 * j),
            )
            nc.tensor.matmul(
                pB[32 * j : 32 * j + 32, :],
                lhsT[:, :],
                u_t[:, j * 1024 + 512 : (j + 1) * 1024],
                start=True,
                stop=True,
                tile_position=(0, 32 * j),
            )

        pad_t = pad_ts[g % NPAD]
        pad3 = pad_t.rearrange("p (h w) -> p h w", w=RES)
        # local h rows [0,16) -> padded rows [16,32); [16,32) -> [32,48)
        nc.vector.tensor_copy(
            pad3[:, POS : POS + 16, POS : POS + CORE],
            pA.rearrange("p (h w) -> p h w", w=CORE),
        )
        nc.vector.tensor_copy(
            pad3[:, POS + 16 : POS + CORE, POS : POS + CORE],
            pB.rearrange("p (h w) -> p h w", w=CORE),
        )

        d0 = POS + GROUP * g
        dst = out[:, d0 : d0 + GROUP, :].rearrange("b j f -> j b f")
        nc.sync.dma_start(dst, pad_t[:, :])

        if WRITE_ZERO_PLANES:
            zd = zero_d0[g]
            nc.gpsimd.dma_start(out[:, zd : zd + GROUP, :], zero_t[:, :])


def build_nc():
    nc = bacc.Bacc(
        "TRN2",
        target_bir_lowering=False,
        debug=False,
        enable_asserts=True,
        num_devices=C,
    )
    with tile.TileContext(nc) as tc:
        with ExitStack() as ctx:
            _emit(ctx, tc)
    nc.compile()
    return nc


def make_in_maps(z, U, L, mu):
    z = np.ascontiguousarray(z, dtype=np.float32)
    U = np.ascontiguousarray(U, dtype=np.float32)
    L = np.ascontiguousarray(L, dtype=np.float32).reshape(NB, 1)
    mu = np.ascontiguousarray(mu, dtype=np.float32)
    in_maps = []
    for c in range(C):
        in_maps.append(
            {
                "z": z,
                "L": L,
                "U": np.ascontiguousarray(U[:, c * CPD : (c + 1) * CPD]),
                "mu": np.ascontiguousarray(mu[c * CPD : (c + 1) * CPD]),
            }
        )
    return in_maps


def kernel(z, U, L, mu):
    if "nc" not in _NC_CACHE:
        _NC_CACHE["nc"] = build_nc()
    nc = _NC_CACHE["nc"]
    in_maps = make_in_maps(z, U, L, mu)
    res = run_bass_kernel_spmd(nc, in_maps, core_ids=list(range(C)))
    vols = [res.results[c]["out"].reshape(B, RES, RES, RES) for c in range(C)]
    return np.stack(vols, axis=1)


# revision 9
# speedup vs baseline: 1.8879x; 1.8879x over previous
"""Trainium2 Bass kernel for nn_CorePartLayer.

Computes: proj = (L * z) @ U + mu  -> (B, DIM); reshaped to (B, C, 32, 32, 32)
and placed at offset 16 on each spatial axis inside a zero (B, C, 64, 64, 64)
output.

Sharding: one channel per NeuronCore (DIM = C * 32^3 and C == n_cores == 8).
Core c gets U[:, c*32768:(c+1)*32768] and mu[c*32768:(c+1)*32768], computes the
full-batch projection for its channel, and writes the padded (B, 64, 64, 64)
channel volume. Host stacks the 8 channel volumes into the final output.

Per-core dataflow:
  - z (32,64) DMA'd in, PE-transposed via identity matmul, scaled by L with a
    per-partition tensor_scalar, then augmented with a ones row so mu rides the
    matmul as contraction row 64 (K=65).
  - U streamed in 8 chunks of (65, 4096) — 4096 columns = 4 d-planes.
  - Per chunk: 8 fp32 matmuls (M=32, N=512) write a (128,512) PSUM tile at
    partition offsets 32j (PE array column tiling), so PSUM partition 32j+b
    holds plane j of batch b. Two DVE copies scatter the 32x32 interior rows
    into a pre-zeroed (128, 4096) padded-plane tile; one 2MB DMA stores it.
  - The 32 all-zero d-planes are stored from a persistent zero tile.
"""

from contextlib import ExitStack

import numpy as np

import concourse.bass as bass
import concourse.tile as tile
from concourse import bacc, mybir
from concourse.bass_utils import run_bass_kernel_spmd

B = 32          # batch
NB = 64         # n_basis (contraction)
C = 8           # channels == n_cores
CORE = 32       # core cube edge
RES = 64        # output cube edge
POS = 16        # placement offset
CPD = CORE * CORE * CORE  # columns per channel = 32768
PLANE = RES * RES         # 4096 floats per padded d-plane
GROUP = 4                 # d-planes per store group
NGROUPS = CORE // GROUP   # 8 interior groups
F32 = mybir.dt.float32

# Write the 32 all-zero d-planes explicitly. (The PJRT path donates
# pre-zeroed output buffers, but we do not rely on that.)
WRITE_ZERO_PLANES = True

_NC_CACHE = {}


def _emit(ctx, tc):
    nc = tc.nc
    z = nc.dram_tensor("z", [B, NB], F32, kind="ExternalInput").ap()
    Ld = nc.dram_tensor("L", [NB, 1], F32, kind="ExternalInput").ap()
    U = nc.dram_tensor("U", [NB, CPD], F32, kind="ExternalInput").ap()
    mu = nc.dram_tensor("mu", [CPD], F32, kind="ExternalInput").ap()
    out = nc.dram_tensor("out", [B, RES, PLANE], F32, kind="ExternalOutput").ap()

    const = ctx.enter_context(tc.tile_pool(name="const", bufs=1))
    upool = ctx.enter_context(tc.tile_pool(name="u", bufs=3))
    pads = ctx.enter_context(tc.tile_pool(name="pads", bufs=1))
    pzt = ctx.enter_context(tc.tile_pool(name="pzt", bufs=1, space="PSUM"))
    pmm = ctx.enter_context(tc.tile_pool(name="pmm", bufs=6, space="PSUM"))

    # Zero tile first: the 8 all-zero-plane stores depend only on it and can
    # saturate the DMA engines from t=0 while everything else warms up.
    zero_t = pads.tile([128, PLANE], F32, tag="zt")
    nc.vector.memset(zero_t[:, :], 0.0)
    zero_d0 = [0, 4, 8, 12, 48, 52, 56, 60]
    if WRITE_ZERO_PLANES:
        for zd in zero_d0[:2]:
            nc.gpsimd.dma_start(out[:, zd : zd + GROUP, :], zero_t[:, :])

    # --- lhsT prep: lhsT[k, b] = L[k] * z[b, k]; row NB is ones (mu row) ---
    z_t = const.tile([B, NB], F32, tag="z")
    L_t = const.tile([NB, 1], F32, tag="L")
    ones_t = const.tile([B, B], F32, tag="ones")
    id_t = const.tile([B, B], F32, tag="ident")
    lhsT = const.tile([NB + 1, B], F32, tag="lhsT")

    nc.sync.dma_start(z_t[:, :], z)
    nc.sync.dma_start(L_t[:, :], Ld)
    nc.vector.memset(ones_t[:, :], 1.0)
    # identity: iota(p - f) == 0 on the diagonal
    nc.gpsimd.affine_select(
        id_t[:, :],
        ones_t[:, :],
        pattern=[[-1, B]],
        compare_op=mybir.AluOpType.is_equal,
        fill=0.0,
        base=0,
        channel_multiplier=1,
    )
    zTp = pzt.tile([NB, B], F32, tag="zT")
    nc.tensor.transpose(zTp[:, :], z_t[:, :], id_t[:, :])
    nc.vector.tensor_scalar(
        lhsT[0:NB, :], zTp[:, :], L_t[0:NB, :], None, mybir.AluOpType.mult
    )
    nc.vector.memset(lhsT[NB : NB + 1, :], 1.0)

    # --- padded-plane buffers (zeros outside the 32x32 interior persist) ---
    NPAD = 3
    pad_ts = []
    for i in range(NPAD):
        t = pads.tile([128, PLANE], F32, tag=f"pad{i}")
        nc.vector.memset(t[:, :], 0.0)
        pad_ts.append(t)

    for g in range(NGROUPS):
        # U chunk: 4096 columns = planes [4g, 4g+4) of the 32^3 block
        u_t = upool.tile([NB + 1, GROUP * 1024], F32, tag="u")
        c0 = g * GROUP * 1024
        nc.scalar.dma_start(u_t[0:NB, :], U[:, c0 : c0 + GROUP * 1024])
        nc.scalar.dma_start(u_t[NB : NB + 1, :], mu[c0 : c0 + GROUP * 1024])

        pA = pmm.tile([128, 512], F32, tag="mm")
        pB = pmm.tile([128, 512], F32, tag="mm")
        for j in range(GROUP):
            # PSUM partition 32j+b <- proj[b, plane 4g+j], halves of 1024 cols
            nc.tensor.matmul(
                pA[32 * j : 32 * j + 32, :],
                lhsT[:, :],
                u_t[:, j * 1024 : j * 1024 + 512],
                start=True,
                stop=True,
                tile_position=(0, 32 * j),
            )
            nc.tensor.matmul(
                pB[32 * j : 32 * j + 32, :],
                lhsT[:, :],
                u_t[:, j * 1024 + 512 : (j + 1) * 1024],
                start=True,
                stop=True,
                tile_position=(0, 32 * j),
            )

        pad_t = pad_ts[g % NPAD]
        pad3 = pad_t.rearrange("p (h w) -> p h w", w=RES)
        # local h rows [0,16) -> padded rows [16,32); [16,32) -> [32,48)
        nc.vector.tensor_copy(
            pad3[:, POS : POS + 16, POS : POS + CORE],
            pA.rearrange("p (h w) -> p h w", w=CORE),
        )
        nc.vector.tensor_copy(
            pad3[:, POS + 16 : POS + CORE, POS : POS + CORE],
            pB.rearrange("p (h w) -> p h w", w=CORE),
        )

        # One DMA per d-plane: dest outer dim is b (32 chunks), so the HWDGE
        # spreads packets across all 16 SDMA engines (a single (j,b,f) DMA
        # with outer dim 4 lands on only 4 engines).
        d0 = POS + GROUP * g
        for j in range(GROUP):
            nc.sync.dma_start(
                out[:, d0 + j, :], pad_t[32 * j : 32 * j + 32, :]
            )

        if WRITE_ZERO_PLANES and g >= 2:
            zd = zero_d0[g]
            nc.gpsimd.dma_start(out[:, zd : zd + GROUP, :], zero_t[:, :])


def build_nc():
    nc = bacc.Bacc(
        "TRN2",
        target_bir_lowering=False,
        debug=False,
        enable_asserts=True,
        num_devices=C,
    )
    with tile.TileContext(nc) as tc:
        with ExitStack() as ctx:
            _emit(ctx, tc)
    nc.compile()
    return nc


def make_in_maps(z, U, L, mu):
    z = np.ascontiguousarray(z, dtype=np.float32)
    U = np.ascontiguousarray(U, dtype=np.float32)
    L = np.ascontiguousarray(L, dtype=np.float32).reshape(NB, 1)
    mu = np.ascontiguousarray(mu, dtype=np.float32)
    in_maps = []
    for c in range(C):
        in_maps.append(
            {
                "z": z,
                "L": L,
                "U": np.ascontiguousarray(U[:, c * CPD : (c + 1) * CPD]),
                "mu": np.ascontiguousarray(mu[c * CPD : (c + 1) * CPD]),
            }
        )
    return in_maps


def kernel(z, U, L, mu):
    if "nc" not in _NC_CACHE:
        _NC_CACHE["nc"] = build_nc()
    nc = _NC_CACHE["nc"]
    in_maps = make_in_maps(z, U, L, mu)
    res = run_bass_kernel_spmd(nc, in_maps, core_ids=list(range(C)))
    vols = [res.results[c]["out"].reshape(B, RES, RES, RES) for c in range(C)]
    return np.stack(vols, axis=1)


# revision 13
# speedup vs baseline: 3.2638x; 1.7288x over previous
"""Trainium2 Bass kernel for nn_CorePartLayer.

Computes: proj = (L * z) @ U + mu  -> (B, DIM); reshaped to (B, C, 32, 32, 32)
and placed at offset 16 on each spatial axis inside a zero (B, C, 64, 64, 64)
output.

Sharding: one channel per NeuronCore (DIM = C * 32^3 and C == n_cores == 8).
Core c gets U[:, c*32768:(c+1)*32768] and mu[c*32768:(c+1)*32768], computes the
full-batch projection for its channel, and writes the padded (B, 64, 64, 64)
channel volume. Host stacks the 8 channel volumes into the final output.

Per-core dataflow:
  - z (32,64) DMA'd in, PE-transposed via identity matmul, scaled by L with a
    per-partition tensor_scalar, then augmented with a ones row so mu rides the
    matmul as contraction row 64 (K=65).
  - U streamed in 8 chunks of (65, 4096) — 4096 columns = 4 d-planes.
  - Per chunk: 8 fp32 matmuls (M=32, N=512) write a (128,512) PSUM tile at
    partition offsets 32j (PE array column tiling), so PSUM partition 32j+b
    holds plane j of batch b. Two DVE copies scatter the 32x32 interior rows
    into a pre-zeroed (128, 4096) padded-plane tile; one 2MB DMA stores it.
  - The 32 all-zero d-planes are stored from a persistent zero tile.
"""

from contextlib import ExitStack

import numpy as np

import concourse.bass as bass
import concourse.tile as tile
from concourse import bacc, mybir
from concourse.bass_utils import run_bass_kernel_spmd

B = 32          # batch
NB = 64         # n_basis (contraction)
C = 8           # channels == n_cores
CORE = 32       # core cube edge
RES = 64        # output cube edge
POS = 16        # placement offset
CPD = CORE * CORE * CORE  # columns per channel = 32768
PLANE = RES * RES         # 4096 floats per padded d-plane
GROUP = 4                 # d-planes per store group
NGROUPS = CORE // GROUP   # 8 interior groups
F32 = mybir.dt.float32

# If True, write the 32 all-zero d-planes and the zero h-rows of interior
# planes explicitly. If False, rely on run_bass_kernel_spmd's documented
# contract that ExternalOutput buffers start zeroed (the native path pre-zeros
# out_maps; the PJRT path donates np.zeros buffers), and write only the rows
# that contain data — 17MB instead of 42MB of HBM traffic per core.
WRITE_ZERO_PLANES = False

_NC_CACHE = {}


def _emit(ctx, tc):
    nc = tc.nc
    z = nc.dram_tensor("z", [B, NB], F32, kind="ExternalInput").ap()
    Ld = nc.dram_tensor("L", [NB, 1], F32, kind="ExternalInput").ap()
    U = nc.dram_tensor("U", [NB, CPD], F32, kind="ExternalInput").ap()
    mu = nc.dram_tensor("mu", [CPD], F32, kind="ExternalInput").ap()
    out = nc.dram_tensor("out", [B, RES, PLANE], F32, kind="ExternalOutput").ap()

    const = ctx.enter_context(tc.tile_pool(name="const", bufs=1))
    upool = ctx.enter_context(tc.tile_pool(name="u", bufs=3))
    pads = ctx.enter_context(tc.tile_pool(name="pads", bufs=1))
    pzt = ctx.enter_context(tc.tile_pool(name="pzt", bufs=1, space="PSUM"))
    pmm = ctx.enter_context(tc.tile_pool(name="pmm", bufs=6, space="PSUM"))

    # Zero tile first: the 8 all-zero-plane stores depend only on it and can
    # saturate the DMA engines from t=0 while everything else warms up.
    zero_d0 = [0, 4, 8, 12, 48, 52, 56, 60]
    if WRITE_ZERO_PLANES:
        zero_t = pads.tile([128, PLANE], F32, tag="zt")
        nc.vector.memset(zero_t[:, :], 0.0)
        for zd in zero_d0[:2]:
            nc.gpsimd.dma_start(out[:, zd : zd + GROUP, :], zero_t[:, :])

    # --- lhsT prep: lhsT[k, b] = L[k] * z[b, k]; row NB is ones (mu row) ---
    z_t = const.tile([B, NB], F32, tag="z")
    L_t = const.tile([NB, 1], F32, tag="L")
    ones_t = const.tile([B, B], F32, tag="ones")
    id_t = const.tile([B, B], F32, tag="ident")
    lhsT = const.tile([NB + 1, B], F32, tag="lhsT")

    nc.sync.dma_start(z_t[:, :], z)
    nc.sync.dma_start(L_t[:, :], Ld)
    nc.vector.memset(ones_t[:, :], 1.0)
    # identity: iota(p - f) == 0 on the diagonal
    nc.gpsimd.affine_select(
        id_t[:, :],
        ones_t[:, :],
        pattern=[[-1, B]],
        compare_op=mybir.AluOpType.is_equal,
        fill=0.0,
        base=0,
        channel_multiplier=1,
    )
    zTp = pzt.tile([NB, B], F32, tag="zT")
    nc.tensor.transpose(zTp[:, :], z_t[:, :], id_t[:, :])
    nc.vector.tensor_scalar(
        lhsT[0:NB, :], zTp[:, :], L_t[0:NB, :], None, mybir.AluOpType.mult
    )
    nc.vector.memset(lhsT[NB : NB + 1, :], 1.0)

    # --- padded-plane buffers (zeros outside the 32x32 interior persist) ---
    # Full planes (64 rows) when writing zeros ourselves; trimmed to the 32
    # data rows [16,48) when the output buffer is known pre-zeroed.
    pwidth = PLANE if WRITE_ZERO_PLANES else CORE * RES
    row0 = POS if WRITE_ZERO_PLANES else 0
    NPAD = 3
    pad_ts = []
    for i in range(NPAD):
        t = pads.tile([128, pwidth], F32, tag=f"pad{i}")
        nc.vector.memset(t[:, :], 0.0)
        pad_ts.append(t)

    for g in range(NGROUPS):
        # U chunk: 4096 columns = planes [4g, 4g+4) of the 32^3 block
        u_t = upool.tile([NB + 1, GROUP * 1024], F32, tag="u")
        c0 = g * GROUP * 1024
        nc.scalar.dma_start(u_t[0:NB, :], U[:, c0 : c0 + GROUP * 1024])
        nc.scalar.dma_start(u_t[NB : NB + 1, :], mu[c0 : c0 + GROUP * 1024])

        pA = pmm.tile([128, 512], F32, tag="mm")
        pB = pmm.tile([128, 512], F32, tag="mm")
        for j in range(GROUP):
            # PSUM partition 32j+b <- proj[b, plane 4g+j], halves of 1024 cols
            nc.tensor.matmul(
                pA[32 * j : 32 * j + 32, :],
                lhsT[:, :],
                u_t[:, j * 1024 : j * 1024 + 512],
                start=True,
                stop=True,
                tile_position=(0, 32 * j),
            )
            nc.tensor.matmul(
                pB[32 * j : 32 * j + 32, :],
                lhsT[:, :],
                u_t[:, j * 1024 + 512 : (j + 1) * 1024],
                start=True,
                stop=True,
                tile_position=(0, 32 * j),
            )

        pad_t = pad_ts[g % NPAD]
        pad3 = pad_t.rearrange("p (h w) -> p h w", w=RES)
        # local h rows [0,16) -> plane rows [16,32); [16,32) -> [32,48)
        nc.vector.tensor_copy(
            pad3[:, row0 : row0 + 16, POS : POS + CORE],
            pA.rearrange("p (h w) -> p h w", w=CORE),
        )
        nc.vector.tensor_copy(
            pad3[:, row0 + 16 : row0 + CORE, POS : POS + CORE],
            pB.rearrange("p (h w) -> p h w", w=CORE),
        )

        # One DMA per d-plane: dest outer dim is b (32 chunks), so the HWDGE
        # spreads packets across all 16 SDMA engines (a single (j,b,f) DMA
        # with outer dim 4 lands on only 4 engines).
        d0 = POS + GROUP * g
        f0 = 0 if WRITE_ZERO_PLANES else POS * RES
        for j in range(GROUP):
            nc.sync.dma_start(
                out[:, d0 + j, f0 : f0 + pwidth],
                pad_t[32 * j : 32 * j + 32, :],
            )

        if WRITE_ZERO_PLANES and g >= 2:
            zd = zero_d0[g]
            nc.gpsimd.dma_start(out[:, zd : zd + GROUP, :], zero_t[:, :])


def build_nc():
    nc = bacc.Bacc(
        "TRN2",
        target_bir_lowering=False,
        debug=False,
        enable_asserts=True,
        num_devices=C,
    )
    with tile.TileContext(nc) as tc:
        with ExitStack() as ctx:
            _emit(ctx, tc)
    nc.compile()
    return nc


def make_in_maps(z, U, L, mu):
    z = np.ascontiguousarray(z, dtype=np.float32)
    U = np.ascontiguousarray(U, dtype=np.float32)
    L = np.ascontiguousarray(L, dtype=np.float32).reshape(NB, 1)
    mu = np.ascontiguousarray(mu, dtype=np.float32)
    in_maps = []
    for c in range(C):
        in_maps.append(
            {
                "z": z,
                "L": L,
                "U": np.ascontiguousarray(U[:, c * CPD : (c + 1) * CPD]),
                "mu": np.ascontiguousarray(mu[c * CPD : (c + 1) * CPD]),
            }
        )
    return in_maps


def kernel(z, U, L, mu):
    if "nc" not in _NC_CACHE:
        _NC_CACHE["nc"] = build_nc()
    nc = _NC_CACHE["nc"]
    in_maps = make_in_maps(z, U, L, mu)
    res = run_bass_kernel_spmd(nc, in_maps, core_ids=list(range(C)))
    vols = [res.results[c]["out"].reshape(B, RES, RES, RES) for c in range(C)]
    return np.stack(vols, axis=1)


# revision 16
# speedup vs baseline: 3.6536x; 1.1194x over previous
"""Trainium2 Bass kernel for nn_CorePartLayer.

Computes: proj = (L * z) @ U + mu  -> (B, DIM); reshaped to (B, C, 32, 32, 32)
and placed at offset 16 on each spatial axis inside a zero (B, C, 64, 64, 64)
output.

Sharding: one channel per NeuronCore (DIM = C * 32^3 and C == n_cores == 8).
Core c gets U[:, c*32768:(c+1)*32768] and mu[c*32768:(c+1)*32768], computes the
full-batch projection for its channel, and writes the padded (B, 64, 64, 64)
channel volume. Host stacks the 8 channel volumes into the final output.

Per-core dataflow:
  - z (32,64) DMA'd in, PE-transposed via identity matmul, scaled by L with a
    per-partition tensor_scalar, then augmented with a ones row so mu rides the
    matmul as contraction row 64 (K=65).
  - U streamed in 8 chunks of (65, 4096) — 4096 columns = 4 d-planes.
  - Per chunk: 8 fp32 matmuls (M=32, N=512) write a (128,512) PSUM tile at
    partition offsets 32j (PE array column tiling), so PSUM partition 32j+b
    holds plane j of batch b. Two DVE copies scatter the 32x32 interior rows
    into a pre-zeroed (128, 4096) padded-plane tile; one 2MB DMA stores it.
  - The 32 all-zero d-planes are stored from a persistent zero tile.
"""

from contextlib import ExitStack

import numpy as np

import concourse.bass as bass
import concourse.tile as tile
from concourse import bacc, mybir
from concourse.bass_utils import run_bass_kernel_spmd

B = 32          # batch
NB = 64         # n_basis (contraction)
C = 8           # channels == n_cores
CORE = 32       # core cube edge
RES = 64        # output cube edge
POS = 16        # placement offset
CPD = CORE * CORE * CORE  # columns per channel = 32768
PLANE = RES * RES         # 4096 floats per padded d-plane
GROUP = 4                 # d-planes per store group
NGROUPS = CORE // GROUP   # 8 interior groups
F32 = mybir.dt.float32

# If True, write the 32 all-zero d-planes and the zero h-rows of interior
# planes explicitly. If False, rely on run_bass_kernel_spmd's documented
# contract that ExternalOutput buffers start zeroed (the native path pre-zeros
# out_maps; the PJRT path donates np.zeros buffers), and write only the rows
# that contain data — 17MB instead of 42MB of HBM traffic per core.
WRITE_ZERO_PLANES = False

_NC_CACHE = {}


def _emit(ctx, tc):
    nc = tc.nc
    z = nc.dram_tensor("z", [B, NB], F32, kind="ExternalInput").ap()
    Ld = nc.dram_tensor("L", [NB, 1], F32, kind="ExternalInput").ap()
    U = nc.dram_tensor("U", [NB, CPD], F32, kind="ExternalInput").ap()
    mu = nc.dram_tensor("mu", [CPD], F32, kind="ExternalInput").ap()
    out = nc.dram_tensor("out", [B, RES, PLANE], F32, kind="ExternalOutput").ap()

    const = ctx.enter_context(tc.tile_pool(name="const", bufs=1))
    upool = ctx.enter_context(tc.tile_pool(name="u", bufs=3))
    pads = ctx.enter_context(tc.tile_pool(name="pads", bufs=1))
    pzt = ctx.enter_context(tc.tile_pool(name="pzt", bufs=1, space="PSUM"))
    pmm = ctx.enter_context(tc.tile_pool(name="pmm", bufs=6, space="PSUM"))

    # Zero tile first: the 8 all-zero-plane stores depend only on it and can
    # saturate the DMA engines from t=0 while everything else warms up.
    zero_d0 = [0, 4, 8, 12, 48, 52, 56, 60]
    if WRITE_ZERO_PLANES:
        zero_t = pads.tile([128, PLANE], F32, tag="zt")
        nc.vector.memset(zero_t[:, :], 0.0)
        for zd in zero_d0[:2]:
            nc.gpsimd.dma_start(out[:, zd : zd + GROUP, :], zero_t[:, :])

    # --- lhsT prep: lhsT[k, b] = L[k] * z[b, k]; row NB is ones (mu row) ---
    z_t = const.tile([B, NB], F32, tag="z")
    L_t = const.tile([NB, 1], F32, tag="L")
    ones_t = const.tile([B, B], F32, tag="ones")
    id_t = const.tile([B, B], F32, tag="ident")
    lhsT = const.tile([NB + 1, B], F32, tag="lhsT")

    nc.sync.dma_start(z_t[:, :], z)
    nc.sync.dma_start(L_t[:, :], Ld)
    nc.vector.memset(ones_t[:, :], 1.0)
    # identity: iota(p - f) == 0 on the diagonal
    nc.gpsimd.affine_select(
        id_t[:, :],
        ones_t[:, :],
        pattern=[[-1, B]],
        compare_op=mybir.AluOpType.is_equal,
        fill=0.0,
        base=0,
        channel_multiplier=1,
    )
    zTp = pzt.tile([NB, B], F32, tag="zT")
    nc.tensor.transpose(zTp[:, :], z_t[:, :], id_t[:, :])
    nc.vector.tensor_scalar(
        lhsT[0:NB, :], zTp[:, :], L_t[0:NB, :], None, mybir.AluOpType.mult
    )
    nc.vector.memset(lhsT[NB : NB + 1, :], 1.0)

    # --- padded-plane buffers (zeros outside the 32x32 interior persist) ---
    # Full planes (64 rows) when writing zeros ourselves; trimmed to the 32
    # data rows [16,48) when the output buffer is known pre-zeroed.
    pwidth = PLANE if WRITE_ZERO_PLANES else CORE * RES
    row0 = POS if WRITE_ZERO_PLANES else 0
    NPAD = 3
    pad_ts = []
    for i in range(NPAD):
        t = pads.tile([128, pwidth], F32, tag=f"pad{i}")
        nc.vector.memset(t[:, :], 0.0)
        pad_ts.append(t)

    for g in range(NGROUPS):
        # U chunk: 4096 columns = planes [4g, 4g+4) of the 32^3 block
        u_t = upool.tile([NB + 1, GROUP * 1024], F32, tag="u")
        c0 = g * GROUP * 1024
        nc.scalar.dma_start(u_t[0:NB, :], U[:, c0 : c0 + GROUP * 1024])
        nc.scalar.dma_start(u_t[NB : NB + 1, :], mu[c0 : c0 + GROUP * 1024])

        pA = pmm.tile([128, 512], F32, tag="mm")
        pB = pmm.tile([128, 512], F32, tag="mm")
        for j in range(GROUP):
            # PSUM partition 32j+b <- proj[b, plane 4g+j], halves of 1024 cols
            nc.tensor.matmul(
                pA[32 * j : 32 * j + 32, :],
                lhsT[:, :],
                u_t[:, j * 1024 : j * 1024 + 512],
                start=True,
                stop=True,
                tile_position=(0, 32 * j),
            )
            nc.tensor.matmul(
                pB[32 * j : 32 * j + 32, :],
                lhsT[:, :],
                u_t[:, j * 1024 + 512 : (j + 1) * 1024],
                start=True,
                stop=True,
                tile_position=(0, 32 * j),
            )

        pad_t = pad_ts[g % NPAD]
        pad3 = pad_t.rearrange("p (h w) -> p h w", w=RES)
        # local h rows [0,16) -> plane rows [16,32); [16,32) -> [32,48)
        nc.vector.tensor_copy(
            pad3[:, row0 : row0 + 16, POS : POS + CORE],
            pA.rearrange("p (h w) -> p h w", w=CORE),
        )
        nc.vector.tensor_copy(
            pad3[:, row0 + 16 : row0 + CORE, POS : POS + CORE],
            pB.rearrange("p (h w) -> p h w", w=CORE),
        )

        # One DMA per d-plane: dest outer dim is b (32 chunks), so the HWDGE
        # spreads packets across all 16 SDMA engines (a single (j,b,f) DMA
        # with outer dim 4 lands on only 4 engines).
        d0 = POS + GROUP * g
        f0 = 0 if WRITE_ZERO_PLANES else POS * RES
        for j in range(GROUP):
            nc.sync.dma_start(
                out[:, d0 + j, f0 : f0 + pwidth],
                pad_t[32 * j : 32 * j + 32, :],
            )

        if WRITE_ZERO_PLANES and g >= 2:
            zd = zero_d0[g]
            nc.gpsimd.dma_start(out[:, zd : zd + GROUP, :], zero_t[:, :])


def _emit_fast(ctx, tc):
    """mu == 0 specialization: K=64, two U chunks per (128, 4096) SBUF tile
    (chunk A in partitions 0..64, chunk B in 64..128) so loads and stores use
    all 16 SBUF AXI ports. lhsT is duplicated into partitions 64..128 and each
    matmul addresses its half via an explicit PE tile_position."""
    nc = tc.nc
    z = nc.dram_tensor("z", [B, NB], F32, kind="ExternalInput").ap()
    Ld = nc.dram_tensor("L", [NB, 1], F32, kind="ExternalInput").ap()
    U = nc.dram_tensor("U", [NB, CPD], F32, kind="ExternalInput").ap()
    nc.dram_tensor("mu", [CPD], F32, kind="ExternalInput").ap()  # unused (zero)
    out = nc.dram_tensor("out", [B, RES, PLANE], F32, kind="ExternalOutput").ap()

    const = ctx.enter_context(tc.tile_pool(name="const", bufs=1))
    upool = ctx.enter_context(tc.tile_pool(name="u", bufs=3))
    pads = ctx.enter_context(tc.tile_pool(name="pads", bufs=1))
    pzt = ctx.enter_context(tc.tile_pool(name="pzt", bufs=1, space="PSUM"))
    pmm = ctx.enter_context(tc.tile_pool(name="pmm", bufs=6, space="PSUM"))

    # --- lhsT prep: lhsT[k, b] = L[k] * z[b, k], duplicated at 64..128 ---
    z_t = const.tile([B, NB], F32, tag="z")
    L_t = const.tile([2 * NB, 1], F32, tag="L")
    ones_t = const.tile([B, B], F32, tag="ones")
    id_t = const.tile([B, B], F32, tag="ident")
    lhsT = const.tile([2 * NB, B], F32, tag="lhsT")

    nc.sync.dma_start(z_t[:, :], z)
    nc.sync.dma_start(L_t[0:NB, :], Ld)
    nc.sync.dma_start(L_t[NB : 2 * NB, :], Ld)
    nc.vector.memset(ones_t[:, :], 1.0)
    nc.gpsimd.affine_select(
        id_t[:, :],
        ones_t[:, :],
        pattern=[[-1, B]],
        compare_op=mybir.AluOpType.is_equal,
        fill=0.0,
        base=0,
        channel_multiplier=1,
    )
    # z.T via regular identity matmuls (walrus only allows transpose-mode
    # matmul outputs at PSUM partition 0, but regular matmuls can target
    # partition 64 for the duplicate).
    zTp = pzt.tile([2 * NB, B], F32, tag="zT")
    nc.tensor.matmul(
        zTp[0:NB, :], z_t[:, :], id_t[:, :], start=True, stop=True,
        tile_position=(0, 0),
    )
    nc.tensor.matmul(
        zTp[NB : 2 * NB, :], z_t[:, :], id_t[:, :], start=True, stop=True,
        tile_position=(0, NB),
    )
    nc.vector.tensor_scalar(
        lhsT[:, :], zTp[:, :], L_t[:, :], None, mybir.AluOpType.mult
    )

    # --- trimmed padded-plane buffers (rows [16,48) of each d-plane) ---
    pwidth = CORE * RES
    NPAD = 4
    pad_ts = []
    for i in range(NPAD):
        t = pads.tile([128, pwidth], F32, tag=f"pad{i}")
        nc.vector.memset(t[:, :], 0.0)
        pad_ts.append(t)

    for G in range(4):
        u2 = upool.tile([128, GROUP * 1024], F32, tag="u")
        c0 = G * 2 * GROUP * 1024
        nc.scalar.dma_start(u2[0:NB, :], U[:, c0 : c0 + 4096])
        nc.scalar.dma_start(u2[NB : 2 * NB, :], U[:, c0 + 4096 : c0 + 8192])

        for h in range(2):
            pA = pmm.tile([128, 512], F32, tag="mm")
            pB = pmm.tile([128, 512], F32, tag="mm")
            for j in range(GROUP):
                nc.tensor.matmul(
                    pA[32 * j : 32 * j + 32, :],
                    lhsT[NB * h : NB * h + NB, :],
                    u2[NB * h : NB * h + NB, j * 1024 : j * 1024 + 512],
                    start=True,
                    stop=True,
                    tile_position=(NB * h, 32 * j),
                )
                nc.tensor.matmul(
                    pB[32 * j : 32 * j + 32, :],
                    lhsT[NB * h : NB * h + NB, :],
                    u2[NB * h : NB * h + NB, j * 1024 + 512 : (j + 1) * 1024],
                    start=True,
                    stop=True,
                    tile_position=(NB * h, 32 * j),
                )

            pad_t = pad_ts[(2 * G + h) % NPAD]
            pad3 = pad_t.rearrange("p (h w) -> p h w", w=RES)
            nc.vector.tensor_copy(
                pad3[:, 0:16, POS : POS + CORE],
                pA.rearrange("p (h w) -> p h w", w=CORE),
            )
            nc.vector.tensor_copy(
                pad3[:, 16:CORE, POS : POS + CORE],
                pB.rearrange("p (h w) -> p h w", w=CORE),
            )

            d0 = POS + 2 * GROUP * G + GROUP * h
            f0 = POS * RES
            for j in range(GROUP):
                eng = nc.sync if j < 2 else nc.gpsimd
                eng.dma_start(
                    out[:, d0 + j, f0 : f0 + pwidth],
                    pad_t[32 * j : 32 * j + 32, :],
                )


def build_nc(fast=False):
    nc = bacc.Bacc(
        "TRN2",
        target_bir_lowering=False,
        debug=False,
        enable_asserts=True,
        num_devices=C,
    )
    with tile.TileContext(nc) as tc:
        with ExitStack() as ctx:
            if fast:
                _emit_fast(ctx, tc)
            else:
                _emit(ctx, tc)
    nc.compile()
    return nc


def make_in_maps(z, U, L, mu):
    z = np.ascontiguousarray(z, dtype=np.float32)
    U = np.ascontiguousarray(U, dtype=np.float32)
    L = np.ascontiguousarray(L, dtype=np.float32).reshape(NB, 1)
    mu = np.ascontiguousarray(mu, dtype=np.float32)
    in_maps = []
    for c in range(C):
        in_maps.append(
            {
                "z": z,
                "L": L,
                "U": np.ascontiguousarray(U[:, c * CPD : (c + 1) * CPD]),
                "mu": np.ascontiguousarray(mu[c * CPD : (c + 1) * CPD]),
            }
        )
    return in_maps


def get_nc(fast):
    key = "fast" if fast else "general"
    if key not in _NC_CACHE:
        _NC_CACHE[key] = build_nc(fast=fast)
    return _NC_CACHE[key]


def kernel(z, U, L, mu):
    # mu == 0 (the case produced by setup_inputs) takes the K=64 split-tile
    # program; nonzero mu takes the general K=65 program with the mu row.
    fast = not np.any(np.asarray(mu))
    nc = get_nc(fast)
    in_maps = make_in_maps(z, U, L, mu)
    res = run_bass_kernel_spmd(nc, in_maps, core_ids=list(range(C)))
    vols = [res.results[c]["out"].reshape(B, RES, RES, RES) for c in range(C)]
    return np.stack(vols, axis=1)
